# revision 1
# baseline (speedup 1.0000x reference)
"""Trainium2 Bass kernel for nn_Attention_45569603010584.

Per-node causal conv attention + FFN over (B=32, C=64, N=207, T=96).
Shards the flattened b*n = 6624 attention-batch dim across 8 cores
(828 each). Each core processes its bns in groups of G=5 (plus a
remainder group of 3), batching all shared-weight matmuls and
elementwise work across the group; only the inherently per-bn
attention matmuls run per bn.

Layout per group (tokens = G*96 columns):
  qk conv   : 2 matmuls (tap2 on x, taps0/1 on shifted copies)
  vT        : per-bn matmul  lhsT=x_bn[64c,96t], rhs=v_wT -> [96t,32h]
  attnT     : per-bn matmul  lhsT=K_bn[32,96], rhs=Q_bn -> [96k,96q]
  mask      : one matmul     lhsT=I96, rhs=(-1000*mask) accumulated
  exp       : ACT on [96, tokens]
  attn_outT : per-bn matmul  lhsT=E_bn[96k,96q], rhs=[vT|1] -> [96q,33]
              (col 32 = softmax denominator)
  normalize : DVE reciprocal + broadcast multiply
  transpose : per-bn PE transpose [96q,32h] -> [32h,96q]
  o-proj    : matmul + residual add;  FFN: 2 matmuls + relu
"""

import numpy as np

B, C, N, T = 32, 64, 207, 96
H = 32
NCORES = 8
BN = B * N              # 6624
BN_CORE = BN // NCORES  # 828
G = 5                   # bns per group
GROUPS = [G] * (BN_CORE // G) + ([BN_CORE % G] if BN_CORE % G else [])
TOK_CORE = BN_CORE * T  # 79488

_CACHE = {}


def _make_tile_context_cls():
    import concourse.mybir as mybir
    from concourse.tile import TileContext, ScopedClock

    class PatchedTileContext(TileContext):
        """The walrus build here rejects instructions carrying more than
        ~2 semaphore waits ("Too many sync wait commands"); TileContext's
        kernel-tail drain aggregates one wait per logical processor onto a
        single Drain. Split them one-per-nop instead."""

        def _split_excess_waits(self):
            """Walrus here allows very few sem waits per TPB instruction;
            move extras onto preceding same-engine nops."""
            nsplit = 0
            for f in self.nc.m.functions:
                for bb in f.blocks:
                    il = bb.instructions
                    out = []
                    for inst in il:
                        si = inst.sync_info
                        if si is not None and len(si.on_wait) > 1:
                            waits = list(si.on_wait)
                            for i, w in enumerate(waits[:-1]):
                                nop = mybir.InstNoOp(
                                    name=f"{inst.name}_wsplit{i}",
                                    engine=inst.engine)
                                nop.sync_info = mybir.SyncInfo(
                                    on_wait=[w], on_update=[])
                                out.append(nop)
                                nsplit += 1
                            inst.sync_info = mybir.SyncInfo(
                                on_wait=waits[-1:],
                                on_update=list(si.on_update))
                        out.append(inst)
                    il[:] = out
            return nsplit

        def _drain_and_barrier(self, tick_clock, wait_clock):
            carrier = self.nc.sync.nop()
            wait_clock.add_sem_waits(
                carrier.ins, ScopedClock({None: tick_clock.global_clock}))
            si = carrier.ins.sync_info
            waits = list(si.on_wait) if si is not None else []
            upd = list(si.on_update) if si is not None else []
            carrier.ins.sync_info = mybir.SyncInfo(on_wait=waits[:1],
                                                   on_update=upd)
            for i in range(1, len(waits)):
                n2 = self.nc.sync.nop()
                n2.ins.sync_info = mybir.SyncInfo(on_wait=waits[i:i + 1],
                                                  on_update=[])
            self.nc.sync.drain()
            self.nc.all_engine_barrier()
            assert self.sems is not None
            popped = self.nc._tile_sem_poison_stack.pop()
            assert popped is self._sem_poison
            self.nc.clear_and_free_semaphores(
                list(self.sems.allocated().values()))
            self.nc.all_engine_barrier()
            self._split_excess_waits()

    return PatchedTileContext


def _build_program(groups=None, tok_total=None, stages=99):
    import concourse.bass as bass
    import concourse.mybir as mybir
    from contextlib import ExitStack

    if groups is None:
        groups = GROUPS
    if tok_total is None:
        tok_total = TOK_CORE
    TOKT = tok_total

    TileContext = _make_tile_context_cls()
    FP = mybir.dt.float32
    FR = mybir.dt.float32r
    nc = bass.Bass()

    xin = nc.dram_tensor("xin", [C, TOKT], FR, kind="ExternalInput")
    wqk2_d = nc.dram_tensor("wqk2", [C, 2 * H], FR, kind="ExternalInput")
    wqk01_d = nc.dram_tensor("wqk01", [2 * C, 2 * H], FR, kind="ExternalInput")
    vwt_d = nc.dram_tensor("vwt", [C, H], FR, kind="ExternalInput")
    owt_d = nc.dram_tensor("owt", [H, C], FR, kind="ExternalInput")
    ff1t_d = nc.dram_tensor("ff1t", [C, C], FR, kind="ExternalInput")
    ff2t_d = nc.dram_tensor("ff2t", [C, C], FR, kind="ExternalInput")
    b1_d = nc.dram_tensor("b1", [C, 1], FP, kind="ExternalInput")
    mneg_d = nc.dram_tensor("mneg", [T, G * T], FR, kind="ExternalInput")
    ident_d = nc.dram_tensor("ident", [T, T], FR, kind="ExternalInput")
    yout = nc.dram_tensor("yout", [C, TOKT], FP, kind="ExternalOutput")

    with TileContext(nc) as tc, ExitStack() as ctx:  # noqa: F841
        const = ctx.enter_context(tc.tile_pool(name="const", bufs=1))

        def load_const(dram, shape, tag, dt=None):
            t = const.tile(shape, dt or FR, tag=tag)
            nc.sync.dma_start(out=t[:], in_=dram[:])
            return t

        wqk2_sb = load_const(wqk2_d, [C, 2 * H], "wqk2")
        wqk01_sb = load_const(wqk01_d, [2 * C, 2 * H], "wqk01")
        vwt_sb = load_const(vwt_d, [C, H], "vwt")
        owt_sb = load_const(owt_d, [H, C], "owt")
        ff1t_sb = load_const(ff1t_d, [C, C], "ff1t")
        ff2t_sb = load_const(ff2t_d, [C, C], "ff2t")
        b1_sb = load_const(b1_d, [C, 1], "b1", dt=FP)
        mneg_sb = load_const(mneg_d, [T, G * T], "mneg")
        ident_sb = load_const(ident_d, [T, T], "ident")

        xp = ctx.enter_context(tc.tile_pool(name="xp", bufs=4))
        sp = ctx.enter_context(tc.tile_pool(name="sp", bufs=3))
        qkp = ctx.enter_context(tc.tile_pool(name="qkp", bufs=3))
        k0p = ctx.enter_context(tc.tile_pool(name="k0p", bufs=3))
        vtp = ctx.enter_context(tc.tile_pool(name="vtp", bufs=3))
        ep = ctx.enter_context(tc.tile_pool(name="ep", bufs=3))
        rxp = ctx.enter_context(tc.tile_pool(name="rxp", bufs=2))
        atnp = ctx.enter_context(tc.tile_pool(name="atnp", bufs=3))
        aop = ctx.enter_context(tc.tile_pool(name="aop", bufs=3))
        ofp = ctx.enter_context(tc.tile_pool(name="ofp", bufs=3))
        h1p = ctx.enter_context(tc.tile_pool(name="h1p", bufs=3))
        outp = ctx.enter_context(tc.tile_pool(name="outp", bufs=3))

        ps_qkv = ctx.enter_context(tc.tile_pool(name="ps_qkv", bufs=1, space="PSUM"))
        ps_vt = ctx.enter_context(tc.tile_pool(name="ps_vt", bufs=1, space="PSUM"))
        ps_at = ctx.enter_context(tc.tile_pool(name="ps_at", bufs=2, space="PSUM"))
        ps_o = ctx.enter_context(tc.tile_pool(name="ps_o", bufs=1, space="PSUM"))
        ps_ao = ctx.enter_context(tc.tile_pool(name="ps_ao", bufs=1, space="PSUM"))
        ps_mlp = ctx.enter_context(tc.tile_pool(name="ps_mlp", bufs=1, space="PSUM"))

        col = 0
        for g in groups:
            tok = g * T

            x_t = xp.tile([C, G * T], FR, tag="x")
            nc.sync.dma_start(out=x_t[:, :tok], in_=xin[:, col:col + tok])

            # shifted copies for conv taps 1 and 0 (left causal pad)
            s_t = sp.tile([2 * C, G * T], FR, tag="s")
            nc.sync.dma_start(out=s_t[0:C, 1:tok], in_=x_t[:, 0:tok - 1])
            nc.sync.dma_start(out=s_t[C:2 * C, 2:tok], in_=x_t[:, 0:tok - 2])
            s3 = s_t.bitcast(FP).rearrange("p (n t) -> p n t", t=T)
            nc.gpsimd.memset(s3[0:C, 0:g, 0:1], 0.0)
            nc.gpsimd.memset(s3[C:2 * C, 0:g, 0:2], 0.0)

            # qk conv: [64 rows: q 0-31, k 32-63] x tokens
            p_qkv = ps_qkv.tile([2 * H, G * T], FP, tag="qkv")
            nc.tensor.matmul(p_qkv[:, :tok], wqk2_sb[:], x_t[:, :tok],
                             start=True, stop=False)
            nc.tensor.matmul(p_qkv[:, :tok], wqk01_sb[:], s_t[:, :tok],
                             start=False, stop=True)
            qk_sb = qkp.tile([2 * H, G * T], FR, tag="qk")
            nc.vector.tensor_copy(qk_sb[:, :tok], p_qkv[:, :tok])
            # K rows to base partition 0 (matmul operands need same base)
            k0_sb = k0p.tile([H, G * T], FR, tag="k0")
            nc.sync.dma_start(out=k0_sb[:, :tok], in_=qk_sb[H:2 * H, :tok])

            if stages < 2:
                nc.sync.dma_start(out=yout[:, col:col + tok], in_=qk_sb[0:C, :tok])
                col += tok
                continue

            # vT per bn: [96t, 32h] each
            p_vt = ps_vt.tile([T, G * H], FP, tag="vt")
            for j in range(g):
                nc.tensor.matmul(p_vt[:, j * H:(j + 1) * H],
                                 x_t[:, j * T:(j + 1) * T], vwt_sb[:],
                                 start=True, stop=True, skip_group_check=True)
            vt_sb = vtp.tile([T, G * (H + 1)], FR, tag="vt_sb")
            vt3 = vt_sb.rearrange("p (n c) -> p n c", c=H + 1)
            nc.vector.tensor_copy(
                vt3[:, 0:g, 0:H],
                p_vt.rearrange("p (n c) -> p n c", c=H)[:, 0:g, :])
            nc.gpsimd.memset(
                vt_sb.bitcast(FP).rearrange("p (n c) -> p n c", c=H + 1)
                [:, 0:g, H:H + 1], 1.0)

            if stages < 3:
                nc.sync.dma_start(out=yout[:, col:col + tok], in_=x_t[:, :tok])
                col += tok
                continue

            # attnT = K^T Q per bn, then -1000*mask accumulated
            p_at = ps_at.tile([T, G * T], FP, tag="at")
            nc.tensor.matmul(p_at[:, :tok], ident_sb[:], mneg_sb[:, :tok],
                             start=True, stop=False, skip_group_check=True)
            for j in range(g):
                nc.tensor.matmul(p_at[:, j * T:(j + 1) * T],
                                 k0_sb[:, j * T:(j + 1) * T],
                                 qk_sb[0:H, j * T:(j + 1) * T],
                                 start=False, stop=(j == g - 1),
                                 skip_group_check=True)
            e_sb = ep.tile([T, G * T], FR, tag="e")
            nc.scalar.activation(out=e_sb[:, :tok], in_=p_at[:, :tok],
                                 func=mybir.ActivationFunctionType.Exp)

            if stages < 4:
                nc.sync.dma_start(out=yout[:, col:col + tok], in_=e_sb[0:C, :tok])
                col += tok
                continue

            # attn_outT (+denominator col) per bn
            p_o = ps_o.tile([T, G * (H + 1)], FP, tag="o")
            for j in range(g):
                nc.tensor.matmul(p_o[:, j * (H + 1):(j + 1) * (H + 1)],
                                 e_sb[:, j * T:(j + 1) * T].bitcast(FP),
                                 vt3[:, j, :].bitcast(FP),
                                 start=True, stop=True, skip_group_check=True)
            o3 = p_o.rearrange("p (n c) -> p n c", c=H + 1)
            rx = rxp.tile([T, G], FP, tag="rx")
            rx3 = rx.rearrange("p (n c) -> p n c", c=1)
            nc.vector.reciprocal(out=rx3[:, 0:g, :], in_=o3[:, 0:g, H:H + 1])
            rx_b = bass.AP(tensor=rx.tensor, offset=rx.offset,
                           ap=[rx.ap[0], [rx.ap[1][0], g], [0, H]])
            atn_sb = atnp.tile([T, G * H], FR, tag="atn")
            atn3 = atn_sb.rearrange("p (n c) -> p n c", c=H)
            nc.vector.tensor_mul(atn3[:, 0:g, :], o3[:, 0:g, 0:H], rx_b)

            if stages < 5:
                nc.sync.dma_start(out=yout[:, col:col + tok], in_=x_t[:, :tok])
                col += tok
                continue

            # transpose each [96q,32h] -> [32h,96q]
            p_ao = ps_ao.tile([H, G * T], FR, tag="ao")
            for j in range(g):
                nc.tensor.transpose(p_ao[:, j * T:(j + 1) * T],
                                    atn_sb[:, j * H:(j + 1) * H],
                                    ident_sb[:])
            ao_sb = aop.tile([H, G * T], FR, tag="ao_sb")
            nc.scalar.copy(out=ao_sb[:, :tok], in_=p_ao[:, :tok])

            if stages < 6:
                nc.sync.dma_start(out=yout[:, col:col + tok], in_=x_t[:, :tok])
                col += tok
                continue

            # out_f = x + o_w @ attn_out
            p_of = ps_mlp.tile([C, G * T], FP, tag="mlp")
            nc.tensor.matmul(p_of[:, :tok], owt_sb[:], ao_sb[:, :tok],
                             start=True, stop=True)
            of_sb = ofp.tile([C, G * T], FR, tag="of_sb")
            nc.vector.tensor_add(of_sb[:, :tok], p_of[:, :tok],
                                 x_t[:, :tok].bitcast(FP))

            if stages < 7:
                nc.sync.dma_start(out=yout[:, col:col + tok], in_=of_sb[:, :tok])
                col += tok
                continue

            # FFN
            p_h1 = ps_mlp.tile([C, G * T], FP, tag="mlp")
            nc.tensor.matmul(p_h1[:, :tok], ff1t_sb[:], of_sb[:, :tok],
                             start=True, stop=True)
            h1_sb = h1p.tile([C, G * T], FR, tag="h1_sb")
            nc.scalar.activation(out=h1_sb[:, :tok], in_=p_h1[:, :tok],
                                 func=mybir.ActivationFunctionType.Relu,
                                 bias=b1_sb[:, 0:1], scale=1.0)
            p_ff = ps_mlp.tile([C, G * T], FP, tag="mlp")
            nc.tensor.matmul(p_ff[:, :tok], ff2t_sb[:], h1_sb[:, :tok],
                             start=True, stop=True)
            out_t = outp.tile([C, G * T], FP, tag="out")
            nc.vector.tensor_add(out_t[:, :tok], p_ff[:, :tok],
                                 of_sb[:, :tok].bitcast(FP))

            nc.sync.dma_start(out=yout[:, col:col + tok], in_=out_t[:, :tok])
            col += tok

    return nc


def _prep_consts(q_w, k_w, v_w, o_w, ff_w1, ff_b1, ff_w2):
    f = np.float32
    wqk2 = np.ascontiguousarray(
        np.concatenate([q_w[:, :, 2], k_w[:, :, 2]], 0).T, dtype=f)
    wqk01 = np.ascontiguousarray(np.concatenate([
        np.concatenate([q_w[:, :, 1], k_w[:, :, 1]], 0).T,
        np.concatenate([q_w[:, :, 0], k_w[:, :, 0]], 0).T], 0), dtype=f)
    vwt = np.ascontiguousarray(v_w.T, dtype=f)
    owt = np.ascontiguousarray(o_w.T, dtype=f)
    ff1t = np.ascontiguousarray(ff_w1.T, dtype=f)
    ff2t = np.ascontiguousarray(ff_w2.T, dtype=f)
    b1 = np.ascontiguousarray(ff_b1.reshape(C, 1), dtype=f)
    mneg1 = np.where(np.arange(T)[:, None] > np.arange(T)[None, :],
                     f(-1000.0), f(0.0)).astype(f)
    mneg = np.ascontiguousarray(np.tile(mneg1, (1, G)))
    ident = np.eye(T, dtype=f)
    return dict(wqk2=wqk2, wqk01=wqk01, vwt=vwt, owt=owt, ff1t=ff1t,
                ff2t=ff2t, b1=b1, mneg=mneg, ident=ident)


def kernel(x, q_w, k_w, v_w, o_w, ff_w1, ff_b1, ff_w2, ff_b2):
    from concourse.bass_utils import run_bass_kernel_spmd

    if "nc" not in _CACHE:
        _CACHE["nc"] = _build_program()
    nc = _CACHE["nc"]

    consts = _prep_consts(q_w, k_w, v_w, o_w, ff_w1, ff_b1, ff_w2)
    xt = np.ascontiguousarray(
        x.transpose(1, 0, 2, 3).reshape(C, BN, T), dtype=np.float32)

    in_maps = []
    for i in range(NCORES):
        xc = np.ascontiguousarray(
            xt[:, i * BN_CORE:(i + 1) * BN_CORE, :].reshape(C, TOK_CORE))
        in_maps.append({"xin": xc, **consts})

    try:
        res = run_bass_kernel_spmd(nc, in_maps, list(range(NCORES)))
    except Exception:
        # a previously wedged device typically clears on retry
        res = run_bass_kernel_spmd(nc, in_maps, list(range(NCORES)))

    out = np.empty((C, BN, T), np.float32)
    for i in range(NCORES):
        out[:, i * BN_CORE:(i + 1) * BN_CORE, :] = \
            res.results[i]["yout"].reshape(C, BN_CORE, T)
    out = out.reshape(C, B, N, T).transpose(1, 0, 2, 3)
    # ff_b2 is added on host (it is all-zeros in this problem's inputs)
    out = out + np.asarray(ff_b2, np.float32)[None, :, None, None]
    return np.ascontiguousarray(out)



# revision 2
# speedup vs baseline: 1.1618x; 1.1618x over previous
"""Trainium2 Bass kernel for nn_Attention_45569603010584 (v2).

Per-node causal conv attention + FFN over (B=32, C=64, N=207, T=96).
Sharding: data-parallel over batch b — core i handles b in [4i, 4i+4).
In the native (b, c, n, t) layout each b-slab x[b] is exactly
[C, N*T] with tokens grouped per bn, so host-side pre/post processing
is zero-copy.

Per core: 4 slabs x 207 bn.  Each slab is processed in 4 macro-tiles
(52/52/52/51 bn).  Within a macro, attention runs in groups of <=5 bn
(PSUM bank = 512 fp32 cols; 5*96 = 480).

Numerics: all matmuls bf16 (1 cyc/row on PE at any width; fp32r would
be 4 cyc/row below 256 cols) except the FFN residual path which stays
fp32.  PSUM accumulation is always fp32.

Layout tricks:
  - conv taps: x is cast into a per-bn zero-padded bf16 layout
    (98 cols/bn, 2 leading zeros); the 3 taps are matmuls of the SAME
    tile at column offsets 0/1/2 accumulating into one PSUM tile.
    No shifted copies, no per-tap DMA.
  - attention: ET[k,q] = K^T Q per bn (lhsT=K needs base partition 0
    == Q's, so K rows are DMA-moved down once per macro).  The mask is
    accumulated into the same PSUM via lhsT=(-1000*I96), rhs=0/1 mask.
  - attn_out[h,q] = matmul(lhsT=[vT|1], rhs=exp(ET)) comes out directly
    in [h, q] layout (no per-bn transpose); the appended ones column
    makes row 32 the softmax denominator.  Normalization multiplies by
    a gpsimd-partition-broadcast reciprocal row.
  - ff biases ride as an extra contraction row ([W.T; b] x [h; 1]).

Engine balance: PE all matmuls; ACT only Exp + copies (single act
table); DVE cast/recip/norm/relu/final-add; GPSIMD memsets,
partition_broadcast, residual add.
"""

import numpy as np

B, C, N, T = 32, 64, 207, 96
H = 32
NCORES = 8
NB = B // NCORES            # 4 slabs (b) per core
NT = N * T                  # 19872 tokens per slab
PBN = T + 2                 # padded cols per bn (2 leading zeros)
MACROS = [(0, 52), (52, 52), (104, 52), (156, 51)]
MMAX = 52
WMAX = MMAX * T             # 4992
PMAX = MMAX * PBN           # 5096
G = 5                       # bn per attention group
GW = G * T                  # 480
GP = G * PBN                # 490

_CACHE = {}

# pool buffer counts — tweakable for timeline-sim sweeps
CFG = dict(sb=3, qk=2, vt=1, at=1, ao=1, rb=1, ml=2)


def _make_tile_context_cls():
    import concourse.mybir as mybir
    from concourse.tile import TileContext, ScopedClock

    class PatchedTileContext(TileContext):
        """The walrus build here rejects instructions carrying more than
        ~2 semaphore waits ("Too many sync wait commands"); TileContext's
        kernel-tail drain aggregates one wait per logical processor onto a
        single Drain. Split them one-per-nop instead."""

        def _split_excess_waits(self):
            nsplit = 0
            for f in self.nc.m.functions:
                for bb in f.blocks:
                    il = bb.instructions
                    out = []
                    for inst in il:
                        si = inst.sync_info
                        if si is not None and len(si.on_wait) > 1:
                            waits = list(si.on_wait)
                            for i, w in enumerate(waits[:-1]):
                                nop = mybir.InstNoOp(
                                    name=f"{inst.name}_wsplit{i}",
                                    engine=inst.engine)
                                nop.sync_info = mybir.SyncInfo(
                                    on_wait=[w], on_update=[])
                                out.append(nop)
                                nsplit += 1
                            inst.sync_info = mybir.SyncInfo(
                                on_wait=waits[-1:],
                                on_update=list(si.on_update))
                        out.append(inst)
                    il[:] = out
            return nsplit

        def _drain_and_barrier(self, tick_clock, wait_clock):
            carrier = self.nc.sync.nop()
            wait_clock.add_sem_waits(
                carrier.ins, ScopedClock({None: tick_clock.global_clock}))
            si = carrier.ins.sync_info
            waits = list(si.on_wait) if si is not None else []
            upd = list(si.on_update) if si is not None else []
            carrier.ins.sync_info = mybir.SyncInfo(on_wait=waits[:1],
                                                   on_update=upd)
            for i in range(1, len(waits)):
                n2 = self.nc.sync.nop()
                n2.ins.sync_info = mybir.SyncInfo(on_wait=waits[i:i + 1],
                                                  on_update=[])
            self.nc.sync.drain()
            self.nc.all_engine_barrier()
            assert self.sems is not None
            popped = self.nc._tile_sem_poison_stack.pop()
            assert popped is self._sem_poison
            self.nc.clear_and_free_semaphores(
                list(self.sems.allocated().values()))
            self.nc.all_engine_barrier()
            self._split_excess_waits()

    return PatchedTileContext


def _groups_of(m):
    gs = [G] * (m // G)
    if m % G:
        gs.append(m % G)
    return gs


def _build_program():
    import concourse.bass as bass
    import concourse.mybir as mybir
    from contextlib import ExitStack

    TileContext = _make_tile_context_cls()
    FP = mybir.dt.float32
    FR = mybir.dt.float32r
    BF = mybir.dt.bfloat16
    AF = mybir.ActivationFunctionType
    nc = bass.Bass()

    xin = nc.dram_tensor("xin", [NB * C, NT], FP, kind="ExternalInput")
    wt21_d = nc.dram_tensor("wt21", [2 * C, 2 * H], BF, kind="ExternalInput")
    wt0_d = nc.dram_tensor("wt0", [C, 2 * H], BF, kind="ExternalInput")
    vwt_d = nc.dram_tensor("vwt", [C, H], BF, kind="ExternalInput")
    owt_d = nc.dram_tensor("owt", [H, C], BF, kind="ExternalInput")
    f1b_d = nc.dram_tensor("f1b", [C, C], FR, kind="ExternalInput")
    f2b_d = nc.dram_tensor("f2b", [C, C], BF, kind="ExternalInput")
    b1_d = nc.dram_tensor("b1", [C, 1], FP, kind="ExternalInput")
    b2_d = nc.dram_tensor("b2", [C, 1], FP, kind="ExternalInput")
    maskc_d = nc.dram_tensor("maskc", [T, GW], BF, kind="ExternalInput")
    yout = nc.dram_tensor("yout", [NB * C, NT], FP, kind="ExternalOutput")

    with TileContext(nc) as tc, ExitStack() as ctx:
        const = ctx.enter_context(tc.tile_pool(name="const", bufs=1))

        def load_const(dram, shape, tag, dt):
            t = const.tile(shape, dt, tag=tag)
            nc.sync.dma_start(out=t[:], in_=dram[:])
            return t

        wt21 = load_const(wt21_d, [2 * C, 2 * H], "wt21", BF)
        vwt = load_const(vwt_d, [C, H], "vwt", BF)
        owt = load_const(owt_d, [H, C], "owt", BF)
        f1b = load_const(f1b_d, [C, C], "f1b", FR)
        f2b = load_const(f2b_d, [C, C], "f2b", BF)
        b1 = load_const(b1_d, [C, 1], "b1", FP)
        b2 = load_const(b2_d, [C, 1], "b2", FP)
        maskc = load_const(maskc_d, [T, GW], "maskc", BF)
        # wt0 sits at partitions 64:128 (pairs with the shifted xb half)
        wt0c = const.tile([2 * C, 2 * H], BF, tag="wt0c")
        nc.sync.dma_start(out=wt0c[C:2 * C, :], in_=wt0_d[:])
        ones1 = const.tile([2, H], FR, tag="ones1")
        nc.gpsimd.memset(ones1.bitcast(FP)[0:2, :], 0.0)
        nc.gpsimd.memset(ones1.bitcast(FP)[0:1, :], 1.0)

        # macro-granular pools
        xp = ctx.enter_context(tc.tile_pool(name="xp", bufs=2))
        xbp = ctx.enter_context(tc.tile_pool(name="xbp", bufs=2))
        qkp = ctx.enter_context(tc.tile_pool(name="qkp", bufs=2))
        k0p = ctx.enter_context(tc.tile_pool(name="k0p", bufs=2))
        outp = ctx.enter_context(tc.tile_pool(name="outp", bufs=2))
        # vt holds a whole macro's per-bn [vT|1] stationaries
        vtp = ctx.enter_context(tc.tile_pool(name="vtp", bufs=2))
        # group-granular pools
        sb = CFG["sb"]
        ep = ctx.enter_context(tc.tile_pool(name="ep", bufs=sb))
        emp = ctx.enter_context(tc.tile_pool(name="emp", bufs=sb))
        rp = ctx.enter_context(tc.tile_pool(name="rp", bufs=sb))
        rbp = ctx.enter_context(tc.tile_pool(name="rbp", bufs=sb))
        aop = ctx.enter_context(tc.tile_pool(name="aop", bufs=sb))
        ofp = ctx.enter_context(tc.tile_pool(name="ofp", bufs=sb + 1))
        h1p = ctx.enter_context(tc.tile_pool(name="h1p", bufs=sb))

        ps_qk = ctx.enter_context(tc.tile_pool(name="ps_qk", bufs=CFG["qk"], space="PSUM"))
        ps_vt = ctx.enter_context(tc.tile_pool(name="ps_vt", bufs=CFG["vt"], space="PSUM"))
        ps_at = ctx.enter_context(tc.tile_pool(name="ps_at", bufs=CFG["at"], space="PSUM"))
        ps_ao = ctx.enter_context(tc.tile_pool(name="ps_ao", bufs=CFG["ao"], space="PSUM"))
        ps_rb = ctx.enter_context(tc.tile_pool(name="ps_rb", bufs=CFG["rb"], space="PSUM"))
        ps_ml = ctx.enter_context(tc.tile_pool(name="ps_ml", bufs=CFG["ml"], space="PSUM"))

        for s in range(NB):
            for (bn0, m) in MACROS:
                W = m * T
                P = m * PBN
                col0 = bn0 * T

                x_t = xp.tile([C, WMAX], FP, tag="x")
                nc.sync.dma_start(out=x_t[:, :W],
                                  in_=xin[s * C:(s + 1) * C, col0:col0 + W])
                x3 = x_t.rearrange("p (n t) -> p n t", t=T)

                xb = xbp.tile([2 * C, PMAX], BF, tag="xb")
                xb3 = xb.rearrange("p (n t) -> p n t", t=PBN)
                nc.gpsimd.memset(xb3[0:C, 0:m, 0:2], 0.0)
                nc.scalar.copy(out=xb3[0:C, 0:m, 2:PBN], in_=x3[:, 0:m, :])
                # rows 64:128 = rows 0:64 shifted right by one column
                nc.gpsimd.memset(xb[C:2 * C, 0:1], 0.0)
                nc.sync.dma_start(out=xb[C:2 * C, 1:P], in_=xb[0:C, 0:P - 1])

                qk_sb = qkp.tile([C, PMAX], BF, tag="qk")
                k0 = k0p.tile([H, PMAX], BF, tag="k0")
                out_t = outp.tile([C, WMAX], FP, tag="out")
                vt = vtp.tile([T, MMAX * (H + 2)], BF, tag="vt")
                vt3 = vt.rearrange("p (n c) -> p n c", c=H + 2)
                nc.gpsimd.memset(vt3[:, 0:m, H:H + 2], 1.0)

                groups = _groups_of(m)

                # ---- phase 1: qk conv taps + vT per group ----
                off = 0
                for g in groups:
                    gp_, gw_ = g * PBN, g * T
                    po = off * PBN
                    p_qk = ps_qk.tile([2 * H, GP], FP, tag="pqk")
                    nc.tensor.matmul(p_qk[:, 0:gp_], wt21[:],
                                     xb[:, po:po + gp_],
                                     start=True, stop=False,
                                     skip_group_check=True)
                    nc.tensor.matmul(p_qk[:, 1:gp_], wt0c[C:2 * C, :],
                                     xb[C:2 * C, po:po + gp_ - 1],
                                     start=False, stop=True,
                                     skip_group_check=True)
                    nc.scalar.copy(out=qk_sb[:, po:po + gp_],
                                   in_=p_qk[:, 0:gp_])

                    p_vt = ps_vt.tile([T, G * H], FP, tag="pvt")
                    for j in range(g):
                        nc.tensor.matmul(
                            p_vt[:, j * H:(j + 1) * H],
                            xb[0:C, po + j * PBN + 2:po + (j + 1) * PBN],
                            vwt[:], start=True, stop=True,
                            skip_group_check=True)
                    nc.scalar.copy(
                        out=vt3[:, off:off + g, 0:H],
                        in_=p_vt.rearrange("p (n c) -> p n c", c=H)[:, 0:g, :])
                    off += g

                # K rows to base partition 0 (matmul operands share a base)
                nc.sync.dma_start(out=k0[:, :P], in_=qk_sb[H:2 * H, :P])

                # ---- phase 2: attention + FFN per group ----
                off = 0
                for g in groups:
                    gw_ = g * T
                    po = off * PBN
                    wo = off * T

                    p_at = ps_at.tile([T, GW], FP, tag="pat")
                    for j in range(g):
                        cs = po + j * PBN + 2
                        nc.tensor.matmul(p_at[:, j * T:(j + 1) * T],
                                         k0[:, cs:cs + T],
                                         qk_sb[0:H, cs:cs + T],
                                         start=(j == 0), stop=(j == g - 1),
                                         skip_group_check=True)
                    e_sb = ep.tile([T, GW], BF, tag="e")
                    nc.scalar.activation(out=e_sb[:, :gw_], in_=p_at[:, :gw_],
                                         func=AF.Exp)
                    em = emp.tile([T, GW], BF, tag="em")
                    nc.gpsimd.tensor_mul(em[:, :gw_], e_sb[:, :gw_],
                                         maskc[:, :gw_])

                    # attn_out rows 0:32, softmax denominator row 32;
                    # the reciprocal row is PE-broadcast into rows 64:96
                    # of the same PSUM bank.
                    p_ao = ps_ao.tile([H + 2, GW], FP, tag="pao")
                    for j in range(g):
                        nc.tensor.matmul(p_ao[0:H + 2, j * T:(j + 1) * T],
                                         vt3[:, off + j, :],
                                         em[:, j * T:(j + 1) * T],
                                         start=True, stop=True,
                                         skip_group_check=True)
                    r = rp.tile([2, GW], FR, tag="r")
                    with nc.allow_low_precision(
                            reason="fp32r reciprocal feeds fp32r matmul"):
                        nc.vector.reciprocal(out=r[0:2, :gw_],
                                             in_=p_ao[H:H + 2, :gw_])
                    p_rb = ps_rb.tile([H, GW], FP, tag="prb")
                    nc.tensor.matmul(p_rb[:, :gw_], ones1[:],
                                     r[:, :gw_], start=True, stop=True,
                                     skip_group_check=True)
                    rb = rbp.tile([H, GW], BF, tag="rb")
                    nc.scalar.copy(out=rb[:, :gw_], in_=p_rb[:, :gw_])
                    ao = aop.tile([H, GW], BF, tag="ao")
                    nc.vector.tensor_mul(ao[:, :gw_], p_ao[0:H, :gw_],
                                         rb[:, :gw_])

                    p_of = ps_ml.tile([C, GW], FP, tag="pml")
                    nc.tensor.matmul(p_of[:, :gw_], owt[:], ao[:, :gw_],
                                     start=True, stop=True)
                    of = ofp.tile([C, GW], FR, tag="of")
                    off32 = of.bitcast(FP)
                    nc.vector.tensor_add(of[:, :gw_], p_of[:, :gw_],
                                         x_t[:, wo:wo + gw_])

                    p_h1 = ps_ml.tile([C, GW], FP, tag="pml")
                    nc.tensor.matmul(p_h1[:, :gw_], f1b[:], of[:, :gw_],
                                     start=True, stop=True)
                    h1 = h1p.tile([C, GW], BF, tag="h1")
                    nc.scalar.activation(out=h1[:, :gw_], in_=p_h1[:, :gw_],
                                         func=AF.Relu, bias=b1[:, 0:1])

                    p_ff = ps_ml.tile([C, GW], FP, tag="pml")
                    nc.tensor.matmul(p_ff[:, :gw_], f2b[:], h1[:, :gw_],
                                     start=True, stop=True)
                    nc.vector.scalar_tensor_tensor(
                        out_t[:, wo:wo + gw_], p_ff[:, :gw_], b2[:, 0:1],
                        off32[:, :gw_], mybir.AluOpType.add,
                        mybir.AluOpType.add)
                    off += g

                nc.sync.dma_start(out=yout[s * C:(s + 1) * C, col0:col0 + W],
                                  in_=out_t[:, :W])

    return nc


def _prep_consts(q_w, k_w, v_w, o_w, ff_w1, ff_b1, ff_w2, ff_b2):
    import ml_dtypes
    bf = ml_dtypes.bfloat16
    f = np.float32

    def qk_tap(k):
        return np.concatenate([q_w[:, :, k], k_w[:, :, k]], 0).T.astype(bf)

    wt21 = np.ascontiguousarray(
        np.concatenate([qk_tap(2), qk_tap(1)], 0))
    wt0 = np.ascontiguousarray(qk_tap(0))
    vwt = np.ascontiguousarray(v_w.T.astype(bf))
    owt = np.ascontiguousarray(o_w.T.astype(bf))
    f1b = np.ascontiguousarray(ff_w1.T, dtype=f)
    f2b = np.ascontiguousarray(ff_w2.T.astype(bf))
    b1 = np.ascontiguousarray(ff_b1.reshape(C, 1), dtype=f)
    b2 = np.ascontiguousarray(ff_b2.reshape(C, 1), dtype=f)
    m1 = (np.arange(T)[:, None] <= np.arange(T)[None, :]).astype(bf)  # k <= q
    maskc = np.ascontiguousarray(np.tile(m1, (1, G)))
    return dict(wt21=wt21, wt0=wt0, vwt=vwt, owt=owt, f1b=f1b,
                f2b=f2b, b1=b1, b2=b2, maskc=maskc)


def kernel(x, q_w, k_w, v_w, o_w, ff_w1, ff_b1, ff_w2, ff_b2):
    from concourse.bass_utils import run_bass_kernel_spmd

    if "nc" not in _CACHE:
        _CACHE["nc"] = _build_program()
    nc = _CACHE["nc"]

    x = np.asarray(x, np.float32)
    if not x.flags.c_contiguous:
        x = np.ascontiguousarray(x)
    consts = _prep_consts(q_w, k_w, v_w, o_w, ff_w1, ff_b1, ff_w2, ff_b2)

    in_maps = []
    for i in range(NCORES):
        xc = x[i * NB:(i + 1) * NB].reshape(NB * C, NT)
        in_maps.append({"xin": xc, **consts})

    try:
        res = run_bass_kernel_spmd(nc, in_maps, list(range(NCORES)))
    except Exception:
        # a previously wedged device typically clears on retry
        res = run_bass_kernel_spmd(nc, in_maps, list(range(NCORES)))

    r0 = res.results[0]["yout"]
    base = r0.base
    full = None
    if (isinstance(base, np.ndarray) and base.dtype == np.float32
            and base.size == B * C * NT and base.flags.c_contiguous):
        full = base.reshape(B, C, N, T)          # zero-copy (axon/pjrt path)
    if full is None:
        full = np.concatenate(
            [res.results[i]["yout"] for i in range(NCORES)],
            axis=0).reshape(B, C, N, T)
    return full
